# revision 1
# baseline (speedup 1.0000x reference)
"""Trainium2 Bass kernel for nn_FDDiscriminator (batched RBF-Gram MMD loss).

Math (matches reference):
  x, y: (B=512, T=128, C=16).  The reference builds 2(T-1)=254 time-pair
  slices; those are the 128 distinct time slices with weights w_t = 1 for
  t in {0, T-1} and 2 otherwise.  Per slice t:
    Kxx = exp(-d(x_t, x_t)/2),  Kxy = exp(-d(x_t, y_t)/2)   (512x512)
  with d[m,n] = |a_m|^2 + |b_n|^2 - 2 a_m.b_n, and
  out = mean_t,w[(sum(Kxx)-N)/(N(N-1))] - 2*mean_t,w[mean(Kxy)].

Device strategy (8 cores, 16 time slices each):
  d comes from K=20 bf16 matmuls with augmented operands (fp32 PSUM):
    lhsT rows = [a^T(16); 1; 1; hi|a|^2; lo|a|^2]
    rhs  rows = [-2 b^T(16); hi(|b|^2 - 2 ln c); lo(...); 1; 1]
  where a = bf16(x), norms are computed FROM the bf16 values and split
  hi+lo bf16 (d is then the exact distance matrix of the bf16-rounded
  inputs to ~2^-17, so the Kxx diagonal stays ~0), and c is a per-gram
  constant folded into the exponent: exp(-0.5(d - 2 ln c)) = c*exp(-d/2).

  Kxx is symmetric: per slice we compute only the 6 upper-triangle
  128x128 blocks (folded c = 2w: counted twice) plus the 4 diagonal
  blocks, themselves split into a 64x64 off-diag quarter (c = 2w) and
  two 64x64 diag sub-blocks (c = w) whose 64-row outputs are stacked
  pairwise at partitions 0/64 of shared PSUM columns -- ACT cost scales
  with free-width only, so the xx window is (128,1152) instead of
  (128,2048).  Kxy is the full gram in one (128,2048) window with
  c = w*2(N-1)/N.  One ScalarE
  activation per window computes exp in place with accum_out giving the
  per-partition sum; the weighted combination then needs no per-slice
  coefficients on device.  Host does the tiny final reduction in f64:
    out = (C_xx - 512*sum(w) - C_xy) / (N(N-1)) / 254.
"""

import numpy as np
import ml_dtypes

BF16 = ml_dtypes.bfloat16

B = 512          # batch (gram size N)
T = 128          # time slices after dedup
C = 16           # channels
K = C + 4        # augmented contraction dim
NCORES = 8
SPT = T // NCORES  # slices per core = 16
NBLK = B // 128    # 4 row blocks per gram
UPPER = [(i, j) for i in range(4) for j in range(4) if i < j]  # 6 pairs

_CACHE = {}


def _build_bass():
    import concourse.bass as bass
    import concourse.bacc as bacc
    import concourse.tile as tile
    import concourse.mybir as mybir

    f32 = mybir.dt.float32
    bf16 = mybir.dt.bfloat16
    Exp = mybir.ActivationFunctionType.Exp
    nc = bacc.Bacc(
        "TRN2", target_bir_lowering=False, debug=False, num_devices=NCORES
    )

    # all four operand tensors packed along the free dim: one DMA per
    # slice costs 20 descriptor-rows instead of 80 (issue cost is per row)
    XIN_d = nc.dram_tensor("XIN", (SPT, K, 4 * B), bf16, kind="ExternalInput").ap()
    ACC_d = nc.dram_tensor("ACC", (1, 2 * SPT), f32, kind="ExternalOutput").ap()

    with tile.TileContext(nc) as tc:
        with (
            tc.tile_pool(name="ins", bufs=4) as inpool,
            tc.tile_pool(name="ps", bufs=1, space="PSUM") as pspool,
            tc.tile_pool(name="acc", bufs=1) as accpool,
        ):
            acc_t = accpool.tile([128, 2 * SPT], f32)
            # ones column for the on-device partition reduction of acc_t:
            # a (1, 2) matmul per slice (hidden under later ACTs) makes the
            # final output DMA a single descriptor instead of 128.
            ones_t = accpool.tile([128, 1], f32)
            nc.gpsimd.memset(ones_t[:], 1.0)
            red_ps = pspool.tile([1, 2 * SPT], f32, tag="red")
            for s in range(SPT):
                xin_t = inpool.tile([K, 4 * B], bf16, tag="xin")
                nc.sync.dma_start(xin_t[:], XIN_d[s])
                l_t = xin_t[:, 0 * B : 1 * B]
                ry_t = xin_t[:, 1 * B : 2 * B]
                ru_t = xin_t[:, 2 * B : 3 * B]
                rd_t = xin_t[:, 3 * B : 4 * B]

                # xy first: needs only the first two DMAs, so the pipeline
                # ramps one DMA earlier.  Full gram (x w*2(N-1)/N).
                pxy = pspool.tile([128, 2048], f32, tag="psxy")
                for i in range(NBLK):
                    nc.tensor.matmul(
                        pxy[:, B * i : B * (i + 1)],
                        lhsT=l_t[:, 128 * i : 128 * (i + 1)],
                        rhs=ry_t,
                        start=True,
                        stop=True,
                    )
                nc.scalar.activation(
                    pxy[:],
                    pxy[:],
                    Exp,
                    scale=-0.5,
                    accum_out=acc_t[:, 2 * s : 2 * s + 1],
                )

                # xx window, (128, 1152):
                #   [0,768):    6 upper-triangle 128-blocks      (x 2w)
                #   [768,896):  4 Q quarters (64x64, rows [0:64) x cols
                #               [64:128) of each diag block), stacked two
                #               per 64-col range at partitions 0/64 (x 2w)
                #   [896,1152): 8 diag 64-sub-blocks, stacked two per
                #               64-col range at partitions 0/64   (x w)
                pxx = pspool.tile([128, 1536], f32, tag="psxx")
                for k, (i, j) in enumerate(UPPER):
                    nc.tensor.matmul(
                        pxx[:, 128 * k : 128 * (k + 1)],
                        lhsT=l_t[:, 128 * i : 128 * (i + 1)],
                        rhs=ru_t[:, 128 * j : 128 * (j + 1)],
                        start=True,
                        stop=True,
                    )
                for i in range(4):
                    half = 64 * (i % 2)
                    col = 768 + 64 * (i // 2)
                    nc.tensor.matmul(
                        pxx[half : half + 64, col : col + 64],
                        lhsT=l_t[:, 128 * i : 128 * i + 64],
                        rhs=ru_t[:, 128 * i + 64 : 128 * (i + 1)],
                        start=True,
                        stop=True,
                    )
                # level 2: each 64-diag splits into a 32x32 quarter (x2w,
                # from ru) + two 32x32 diags (xw, from rd); quarters and
                # diags stack 4-high at partitions 0/32/64/96
                for i in range(4):
                    for h in range(2):
                        q = 2 * i + h  # 0..7
                        base = 128 * i + 64 * h
                        nc.tensor.matmul(
                            pxx[32 * (q % 4) : 32 * (q % 4) + 32,
                                896 + 32 * (q // 4) : 928 + 32 * (q // 4)],
                            lhsT=l_t[:, base : base + 32],
                            rhs=ru_t[:, base + 32 : base + 64],
                            start=True,
                            stop=True,
                            tile_position=(0, 32 * (q % 4)),
                        )
                for i in range(4):
                    for h in range(4):
                        d = 4 * i + h  # 0..15
                        base = 128 * i + 32 * h
                        nc.tensor.matmul(
                            pxx[32 * (d % 4) : 32 * (d % 4) + 32,
                                960 + 32 * (d // 4) : 992 + 32 * (d // 4)],
                            lhsT=l_t[:, base : base + 32],
                            rhs=rd_t[:, base : base + 32],
                            start=True,
                            stop=True,
                            tile_position=(0, 32 * (d % 4)),
                        )
                nc.scalar.activation(
                    pxx[:, 0:1088],
                    pxx[:, 0:1088],
                    Exp,
                    scale=-0.5,
                    accum_out=acc_t[:, 2 * s + 1 : 2 * s + 2],
                )
                nc.tensor.matmul(
                    red_ps[:, 2 * s : 2 * s + 2],
                    lhsT=ones_t[:],
                    rhs=acc_t[:, 2 * s : 2 * s + 2],
                    start=True,
                    stop=True,
                )
            accs_t = accpool.tile([1, 2 * SPT], f32)
            nc.scalar.copy(accs_t[:], red_ps[:])
            nc.sync.dma_start(ACC_d, accs_t[:])

    nc.compile()
    return nc


def _split_hi_lo(v):
    hi = v.astype(BF16)
    lo = (v - hi.astype(np.float32)).astype(BF16)
    return hi, lo


def _rhs(neg2T, sq_shift):
    """neg2T: (SPT, C, B) bf16; sq_shift: (SPT, B) f32 -> (SPT, K, B) bf16."""
    R = np.empty((SPT, K, B), BF16)
    R[:, :C] = neg2T
    R[:, C], R[:, C + 1] = _split_hi_lo(sq_shift)
    R[:, C + 2] = np.asarray(1.0, BF16)
    R[:, C + 3] = np.asarray(1.0, BF16)
    return R


def _prep_core(xs, ys, w):
    """xs, ys: (B, SPT, C) f32; w: (SPT,) weights -> L, RU, RD, RY bf16."""
    xb = xs.astype(BF16)
    yb = ys.astype(BF16)
    xT = np.ascontiguousarray(xb.transpose(1, 2, 0))  # (SPT, C, B)
    yT = np.ascontiguousarray(yb.transpose(1, 2, 0))
    nxT = (-2.0 * xT.astype(np.float32)).astype(BF16)  # exact 2x scale
    nyT = (-2.0 * yT.astype(np.float32)).astype(BF16)
    sqx = (xb.astype(np.float32) ** 2).sum(axis=2).T  # (SPT, B) f32
    sqy = (yb.astype(np.float32) ** 2).sum(axis=2).T

    L = np.empty((SPT, K, B), BF16)
    L[:, :C] = xT
    L[:, C] = np.asarray(1.0, BF16)
    L[:, C + 1] = np.asarray(1.0, BF16)
    L[:, C + 2], L[:, C + 3] = _split_hi_lo(sqx)

    c_u = 2.0 * w  # upper blocks counted twice
    c_d = w
    c_y = w * (2.0 * (B - 1) / B)
    shift = lambda cs: (2.0 * np.log(cs))[:, None].astype(np.float32)
    RU = _rhs(nxT, sqx - shift(c_u))
    RD = _rhs(nxT, sqx - shift(c_d))
    RY = _rhs(nyT, sqy - shift(c_y))
    # packed to match the device layout: one DMA per slice
    return np.ascontiguousarray(np.concatenate([L, RY, RU, RD], axis=2))


def _run(x, y, trace=False, **kw):
    from concourse.bass_utils import run_bass_kernel_spmd

    if "nc" not in _CACHE:
        _CACHE["nc"] = _build_bass()
    nc = _CACHE["nc"]

    w = np.full(T, 2.0)
    w[0] = w[T - 1] = 1.0
    in_maps = []
    for c in range(NCORES):
        sl = slice(c * SPT, (c + 1) * SPT)
        in_maps.append({"XIN": _prep_core(x[:, sl, :], y[:, sl, :], w[sl])})

    return run_bass_kernel_spmd(
        nc, in_maps, list(range(NCORES)), trace=trace, **kw
    )


def _run_with_retries(x, y, trace=False, _trace_kw=None):
    """First execution of a freshly-loaded NEFF occasionally dies with
    NRT_EXEC_UNIT_UNRECOVERABLE; retry, resetting the jax backend in
    between, then fall back to a fresh subprocess."""
    import time as _time

    last = None
    for attempt in range(3):
        try:
            return _run(x, y, trace=trace, **(_trace_kw or {}))
        except Exception as e:  # noqa: BLE001
            last = e
            try:
                import jax

                jax.clear_caches()
                jax.clear_backends()
            except Exception:
                pass
            _time.sleep(2.0)
    # subprocess fallback: fresh process, fresh device session
    import os
    import pickle
    import subprocess
    import sys
    import tempfile

    kdir = os.path.dirname(os.path.abspath(__file__))
    with tempfile.TemporaryDirectory() as td:
        inp = os.path.join(td, "io.pkl")
        with open(inp, "wb") as f:
            pickle.dump({"x": x, "y": y}, f)
        code = (
            "import pickle, sys; sys.path.insert(0, %r); import kernel as km; "
            "d = pickle.load(open(%r, 'rb')); "
            "r = km.kernel(d['x'], d['y']); "
            "pickle.dump(r, open(%r, 'wb'))"
            % (kdir, inp, inp + ".out")
        )
        for attempt in range(2):
            p = subprocess.run(
                [sys.executable, "-c", code], capture_output=True, timeout=1800
            )
            if p.returncode == 0 and os.path.exists(inp + ".out"):
                with open(inp + ".out", "rb") as f:
                    return pickle.load(f)
    raise last


def kernel(x, y, _trace=False, _trace_kw=None):
    x = np.asarray(x, np.float32)
    y = np.asarray(y, np.float32)
    res = _run_with_retries(x, y, trace=_trace, _trace_kw=_trace_kw)
    if isinstance(res, np.floating | np.ndarray):
        return res  # came from the subprocess fallback, already reduced

    c_xx = 0.0
    c_xy = 0.0
    for c in range(NCORES):
        acc = np.asarray(res.results[c]["ACC"], np.float64)  # (128, 2*SPT)
        sums = acc.sum(axis=0)
        c_xy += sums[0::2].sum()  # xy windows run first per slice
        c_xx += sums[1::2].sum()
    out = (c_xx - 512.0 * 254.0 - c_xy) / (B * (B - 1)) / 254.0
    if _trace:
        kernel.last_results = res
    return np.float32(out)



# revision 6
# speedup vs baseline: 1.4829x; 1.4829x over previous
"""Trainium2 Bass kernel for nn_FDDiscriminator (batched RBF-Gram MMD loss).

Math (matches reference):
  x, y: (B=512, T=128, C=16).  The reference builds 2(T-1)=254 time-pair
  slices; those are the 128 distinct time slices with weights w_t = 1 for
  t in {0, T-1} and 2 otherwise.  Per slice t:
    Kxx = exp(-d(x_t, x_t)/2),  Kxy = exp(-d(x_t, y_t)/2)   (512x512)
  with d[m,n] = |a_m|^2 + |b_n|^2 - 2 a_m.b_n, and
  out = mean_t,w[(sum(Kxx)-N)/(N(N-1))] - 2*mean_t,w[mean(Kxy)].

Device strategy (8 cores, 16 time slices each):
  d comes from K=20 bf16 matmuls with augmented operands (fp32 PSUM):
    lhsT rows = [a^T(16); 1; 1; hi|a|^2; lo|a|^2]
    rhs  rows = [-2 b^T(16); hi(|b|^2 - 2 ln c); lo(...); 1; 1]
  where a = bf16(x), norms are computed FROM the bf16 values and split
  hi+lo bf16, and c is a per-gram constant folded into the exponent:
  exp(-0.5(d - 2 ln c)) = c*exp(-d/2).

  exp is SPLIT across two engines to break the ScalarE bottleneck:
   - ScalarE (ACT): exact exp via activation on the leading A_XY cols of
     the xy window and the leading A_XX cols of the xx window (which are
     laid out diag-blocks-first, so the Kxx diagonal goes through exact
     exp and the host subtracts exactly 512*254).
   - VectorE (DVE): the remaining cols via a Schraudolph exp: one
     tensor_scalar computes int32(d*C1 + C2) whose int32 bit pattern IS
     the fp32 approximation of c*exp(-d/2) (max err ~4%, mean ~4e-4 with
     the tuned C2; the final loss averages ~8M of these).  Because y has
     ulp 64 at 2^30 the float->int cast is exact under any rounding mode.
  Both engines write (bits of) fp32 values into shared SBUF tiles; the
  PE reduces each 128-col chunk with a stationary-weights fp32 matmul
  against a ones column (cost ~ 4 PE cycles per chunk) accumulated into
  per-slice PSUM columns; a final ones-matmul collapses partitions and
  one small DMA returns (1, 2*SPT) per core.  Host combine:
    out = (C_xx - 512*254 - C_xy) / (N(N-1)) / 254.
"""

import numpy as np
import ml_dtypes

BF16 = ml_dtypes.bfloat16

B = 512          # batch (gram size N)
T = 128          # time slices after dedup
C = 16           # channels
K = C + 4        # augmented contraction dim
NCORES = 8
SPT = T // NCORES  # slices per core = 16
XX = 1088          # xx window cols: 128 d32 + 64 q32 + 128 Q64 + 768 upper
UPPER = [(i, j) for i in range(4) for j in range(4) if i < j]  # 6 pairs

# Schraudolph: int32(d*C1 + C2) bit-viewed as fp32 ~= exp(-d/2).
# C2 tuned for zero mean relative error under uniform exponent fraction.
C1 = float(np.float32(-0.5 * np.log2(np.e) * (1 << 23)))
C2 = float(np.float32((127.0 - 0.05752) * (1 << 23)))

# Four PSUM window tiles, one per consumer instruction (concurrent readers
# of one PSUM tile serialize in the tile framework, so every concurrently-
# running exp instruction gets its own tile):
#   T1 "A" [128,1024] xy[0:1024)    -> ScalarE exact exp
#   T2 "V" [128,1024] xy[1024:2048) -> DVE Schraudolph
#   T4 "A" [128, 640] xx: 16 d32 diag blocks (x w, diag exact on ACT),
#                         4 Q64 quarters, uppers 0-2        (x 2w)
#   T3 "V" [128, 448] xx: 8 q32 quarters, uppers 3-5        (x 2w)
_CACHE = {}


def _build_bass():
    import concourse.bass as bass
    import concourse.bacc as bacc
    import concourse.tile as tile
    import concourse.mybir as mybir

    f32 = mybir.dt.float32
    i32 = mybir.dt.int32
    bf16 = mybir.dt.bfloat16
    Exp = mybir.ActivationFunctionType.Exp
    Mult = mybir.AluOpType.mult
    Add = mybir.AluOpType.add
    nc = bacc.Bacc(
        "TRN2", target_bir_lowering=False, debug=False, num_devices=NCORES
    )

    XIN_d = nc.dram_tensor("XIN", (SPT, K, 4 * B), bf16, kind="ExternalInput").ap()
    ACC_d = nc.dram_tensor("ACC", (1, 2 * SPT), f32, kind="ExternalOutput").ap()

    with tile.TileContext(nc) as tc:
        with (
            tc.tile_pool(name="ins", bufs=4) as inpool,
            tc.tile_pool(name="ps", bufs=1, space="PSUM") as pspool,
            tc.tile_pool(name="es", bufs=2) as epool,
            tc.tile_pool(name="acc", bufs=1) as accpool,
        ):
            ones_t = accpool.tile([128, 1], f32)
            nc.gpsimd.memset(ones_t[:], 1.0)
            p1 = pspool.tile([128, 1024], f32, tag="p1")
            p2 = pspool.tile([128, 1024], f32, tag="p2")
            p4 = pspool.tile([128, 640], f32, tag="p4")
            p3 = pspool.tile([128, 448], f32, tag="p3")
            red_ps = pspool.tile([128, 2 * SPT], f32, tag="red")
            # (tile, engine, is_xx, used cols)
            segs = [(p1, "A", False, 1024), (p2, "V", False, 1024),
                    (p4, "A", True, 640), (p3, "V", True, 448)]
            etiles = []
            for s in range(SPT):
                xin_t = inpool.tile([K, 4 * B], bf16, tag="xin")
                nc.sync.dma_start(xin_t[:], XIN_d[s])
                l_t = xin_t[:, 0 * B : 1 * B]
                ry_t = xin_t[:, 1 * B : 2 * B]
                ru_t = xin_t[:, 2 * B : 3 * B]
                rd_t = xin_t[:, 3 * B : 4 * B]

                # xy gram: 8 x 128-wide matmuls into each of T1, T2
                for j in range(16):
                    pt = p1 if j < 8 else p2
                    nc.tensor.matmul(
                        pt[:, 128 * (j % 8) : 128 * (j % 8 + 1)],
                        lhsT=l_t[:, 128 * (j // 4) : 128 * (j // 4 + 1)],
                        rhs=ry_t[:, 128 * (j % 4) : 128 * (j % 4 + 1)],
                        start=True,
                        stop=True,
                    )
                # T4: d32 diag sub-blocks at [0:128), 4-high (x w)
                for i in range(4):
                    for h in range(4):
                        d = 4 * i + h
                        base = 128 * i + 32 * h
                        nc.tensor.matmul(
                            p4[32 * (d % 4) : 32 * (d % 4) + 32,
                               32 * (d // 4) : 32 * (d // 4) + 32],
                            lhsT=l_t[:, base : base + 32],
                            rhs=rd_t[:, base : base + 32],
                            start=True,
                            stop=True,
                            tile_position=(0, 32 * (d % 4)),
                        )
                # T4: Q64 quarters at [128:256), 2-high (x 2w)
                for i in range(4):
                    half = 64 * (i % 2)
                    col = 128 + 64 * (i // 2)
                    nc.tensor.matmul(
                        p4[half : half + 64, col : col + 64],
                        lhsT=l_t[:, 128 * i : 128 * i + 64],
                        rhs=ru_t[:, 128 * i + 64 : 128 * (i + 1)],
                        start=True,
                        stop=True,
                    )
                # T4: uppers 0-2 at [256:640); T3: q32 at [0:64) + uppers
                # 3-5 at [64:448)
                for i in range(4):
                    for h in range(2):
                        q = 2 * i + h
                        base = 128 * i + 64 * h
                        nc.tensor.matmul(
                            p3[32 * (q % 4) : 32 * (q % 4) + 32,
                               32 * (q // 4) : 32 * (q // 4) + 32],
                            lhsT=l_t[:, base : base + 32],
                            rhs=ru_t[:, base + 32 : base + 64],
                            start=True,
                            stop=True,
                            tile_position=(0, 32 * (q % 4)),
                        )
                for k, (i, j) in enumerate(UPPER):
                    pt, col = (p4, 256 + 128 * k) if k < 3 else (p3, 64 + 128 * (k - 3))
                    nc.tensor.matmul(
                        pt[:, col : col + 128],
                        lhsT=l_t[:, 128 * i : 128 * (i + 1)],
                        rhs=ru_t[:, 128 * j : 128 * (j + 1)],
                        start=True,
                        stop=True,
                    )

                # consumers: one instruction per window tile
                etl = []
                for idx, (pt, eng, _, w) in enumerate(segs):
                    e_t = epool.tile([128, w], f32, tag=f"e{idx}")
                    if eng == "A":
                        nc.scalar.activation(e_t[:], pt[:, 0:w], Exp, scale=-0.5)
                    else:
                        nc.vector.tensor_scalar(
                            e_t[:].bitcast(i32), pt[:, 0:w], C1, C2, Mult, Add)
                    etl.append(e_t)
                etiles.append(etl)

                if s > 0:
                    _reduce(nc, segs, etiles[s - 1], red_ps, ones_t, s - 1)
            _reduce(nc, segs, etiles[SPT - 1], red_ps, ones_t, SPT - 1)

            acc_sb = accpool.tile([128, 2 * SPT], f32)
            nc.vector.tensor_copy(acc_sb[:], red_ps[:])
            nc.tensor.matmul(red_ps[0:1, :], lhsT=ones_t[:], rhs=acc_sb[:],
                             start=True, stop=True, skip_group_check=True)
            accs_t = accpool.tile([1, 2 * SPT], f32)
            nc.vector.tensor_copy(accs_t[:], red_ps[0:1, :])
            nc.sync.dma_start(ACC_d, accs_t[:])

    nc.compile()
    return nc


def _reduce(nc, segs, etl, red_ps, ones_t, s):
    """Sum each exp tile into per-slice red columns (xy vs xx) via
    stationary-weights fp32 matmuls: ~4 PE cycles per 128-col chunk."""
    for col in (False, True):  # xy first, then xx
        sel = []
        for (pt, eng, is_xx, w), e_t in zip(segs, etl):
            if is_xx != col:
                continue
            lo = 0
            while lo < w:
                hi = min(lo + 128, w)
                sel.append((e_t, lo, hi))
                lo = hi
        for n, (e_t, lo, hi) in enumerate(sel):
            nc.tensor.matmul(
                red_ps[0 : hi - lo, 2 * s + int(col) : 2 * s + int(col) + 1],
                lhsT=e_t[:, lo:hi],
                rhs=ones_t[:],
                start=(n == 0),
                stop=(n == len(sel) - 1),
            )


def _split_hi_lo(v):
    hi = v.astype(BF16)
    lo = (v - hi.astype(np.float32)).astype(BF16)
    return hi, lo


def _rhs(neg2T, sq_shift):
    """neg2T: (SPT, C, B) bf16; sq_shift: (SPT, B) f32 -> (SPT, K, B) bf16."""
    R = np.empty((SPT, K, B), BF16)
    R[:, :C] = neg2T
    R[:, C], R[:, C + 1] = _split_hi_lo(sq_shift)
    R[:, C + 2] = np.asarray(1.0, BF16)
    R[:, C + 3] = np.asarray(1.0, BF16)
    return R


def _prep_core(xs, ys, w):
    """xs, ys: (B, SPT, C) f32; w: (SPT,) weights -> packed operands."""
    xb = xs.astype(BF16)
    yb = ys.astype(BF16)
    xT = np.ascontiguousarray(xb.transpose(1, 2, 0))  # (SPT, C, B)
    yT = np.ascontiguousarray(yb.transpose(1, 2, 0))
    nxT = (-2.0 * xT.astype(np.float32)).astype(BF16)  # exact 2x scale
    nyT = (-2.0 * yT.astype(np.float32)).astype(BF16)
    sqx = (xb.astype(np.float32) ** 2).sum(axis=2).T  # (SPT, B) f32
    sqy = (yb.astype(np.float32) ** 2).sum(axis=2).T

    L = np.empty((SPT, K, B), BF16)
    L[:, :C] = xT
    L[:, C] = np.asarray(1.0, BF16)
    L[:, C + 1] = np.asarray(1.0, BF16)
    L[:, C + 2], L[:, C + 3] = _split_hi_lo(sqx)

    c_u = 2.0 * w  # off-diag-pair blocks counted twice
    c_d = w
    c_y = w * (2.0 * (B - 1) / B)
    shift = lambda cs: (2.0 * np.log(cs))[:, None].astype(np.float32)
    RU = _rhs(nxT, sqx - shift(c_u))
    RD = _rhs(nxT, sqx - shift(c_d))
    RY = _rhs(nyT, sqy - shift(c_y))
    # packed to match the device layout: one DMA per slice
    return np.ascontiguousarray(np.concatenate([L, RY, RU, RD], axis=2))


def _run(x, y, trace=False, **kw):
    from concourse.bass_utils import run_bass_kernel_spmd

    if "nc" not in _CACHE:
        _CACHE["nc"] = _build_bass()
    nc = _CACHE["nc"]

    w = np.full(T, 2.0)
    w[0] = w[T - 1] = 1.0
    in_maps = []
    for c in range(NCORES):
        sl = slice(c * SPT, (c + 1) * SPT)
        in_maps.append({"XIN": _prep_core(x[:, sl, :], y[:, sl, :], w[sl])})

    return run_bass_kernel_spmd(
        nc, in_maps, list(range(NCORES)), trace=trace, **kw
    )


def _run_with_retries(x, y, trace=False, _trace_kw=None):
    """First execution of a freshly-loaded NEFF occasionally dies with
    NRT_EXEC_UNIT_UNRECOVERABLE; retry, resetting the jax backend in
    between, then fall back to a fresh subprocess."""
    import time as _time

    last = None
    for attempt in range(3):
        try:
            return _run(x, y, trace=trace, **(_trace_kw or {}))
        except Exception as e:  # noqa: BLE001
            last = e
            try:
                import jax

                jax.clear_caches()
                jax.clear_backends()
            except Exception:
                pass
            _time.sleep(2.0)
    # subprocess fallback: fresh process, fresh device session
    import os
    import pickle
    import subprocess
    import sys
    import tempfile

    kdir = os.path.dirname(os.path.abspath(__file__))
    with tempfile.TemporaryDirectory() as td:
        inp = os.path.join(td, "io.pkl")
        with open(inp, "wb") as f:
            pickle.dump({"x": x, "y": y}, f)
        code = (
            "import pickle, sys; sys.path.insert(0, %r); import kernel as km; "
            "d = pickle.load(open(%r, 'rb')); "
            "r = km.kernel(d['x'], d['y']); "
            "pickle.dump(r, open(%r, 'wb'))"
            % (kdir, inp, inp + ".out")
        )
        for attempt in range(2):
            p = subprocess.run(
                [sys.executable, "-c", code], capture_output=True, timeout=1800
            )
            if p.returncode == 0 and os.path.exists(inp + ".out"):
                with open(inp + ".out", "rb") as f:
                    return pickle.load(f)
    raise last


def kernel(x, y, _trace=False, _trace_kw=None):
    x = np.asarray(x, np.float32)
    y = np.asarray(y, np.float32)
    res = _run_with_retries(x, y, trace=_trace, _trace_kw=_trace_kw)
    if isinstance(res, np.floating | np.ndarray):
        return res  # came from the subprocess fallback, already reduced

    c_xx = 0.0
    c_xy = 0.0
    for c in range(NCORES):
        acc = np.asarray(res.results[c]["ACC"], np.float64)  # (1, 2*SPT)
        c_xy += acc[0, 0::2].sum()
        c_xx += acc[0, 1::2].sum()
    out = (c_xx - 512.0 * 254.0 - c_xy) / (B * (B - 1)) / 254.0
    if _trace:
        kernel.last_results = res
    return np.float32(out)


# revision 13
# speedup vs baseline: 1.5239x; 1.0276x over previous
"""Trainium2 Bass kernel for nn_FDDiscriminator (batched RBF-Gram MMD loss).

Math (matches reference):
  x, y: (B=512, T=128, C=16).  The reference builds 2(T-1)=254 time-pair
  slices; those are the 128 distinct time slices with weights w_t = 1 for
  t in {0, T-1} and 2 otherwise.  Per slice t:
    Kxx = exp(-d(x_t, x_t)/2),  Kxy = exp(-d(x_t, y_t)/2)   (512x512)
  with d[m,n] = |a_m|^2 + |b_n|^2 - 2 a_m.b_n, and
  out = mean_t,w[(sum(Kxx)-N)/(N(N-1))] - 2*mean_t,w[mean(Kxy)].

Device strategy (8 cores, 16 time slices each):
  d comes from K=20 bf16 matmuls with augmented operands (fp32 PSUM):
    lhsT rows = [a^T(16); 1; 1; hi|a|^2; lo|a|^2]
    rhs  rows = [-2 b^T(16); hi(|b|^2 - 2 ln c); lo(...); 1; 1]
  where a = bf16(x), norms are computed FROM the bf16 values and split
  hi+lo bf16, and c is a per-gram constant folded into the exponent:
  exp(-0.5(d - 2 ln c)) = c*exp(-d/2).

  exp is SPLIT across two engines to break the ScalarE bottleneck:
   - ScalarE (ACT): exact exp via activation on the leading A_XY cols of
     the xy window and the leading A_XX cols of the xx window (which are
     laid out diag-blocks-first, so the Kxx diagonal goes through exact
     exp and the host subtracts exactly 512*254).
   - VectorE (DVE): the remaining cols via a Schraudolph exp: one
     tensor_scalar computes int32(d*C1 + C2) whose int32 bit pattern IS
     the fp32 approximation of c*exp(-d/2) (max err ~4%, mean ~4e-4 with
     the tuned C2; the final loss averages ~8M of these).  Because y has
     ulp 64 at 2^30 the float->int cast is exact under any rounding mode.
  Both engines write (bits of) fp32 values into shared SBUF tiles; the
  PE reduces each 128-col chunk with a stationary-weights fp32 matmul
  against a ones column (cost ~ 4 PE cycles per chunk) accumulated into
  per-slice PSUM columns; a final ones-matmul collapses partitions and
  one small DMA returns (1, 2*SPT) per core.  Host combine:
    out = (C_xx - 512*254 - C_xy) / (N(N-1)) / 254.
"""

import numpy as np
import ml_dtypes

BF16 = ml_dtypes.bfloat16

B = 512          # batch (gram size N)
T = 128          # time slices after dedup
C = 16           # channels
K = C + 4        # augmented contraction dim
NCORES = 8
SPT = T // NCORES  # slices per core = 16
XX = 1088          # xx window cols: 128 d32 + 64 q32 + 128 Q64 + 768 upper
UPPER = [(i, j) for i in range(4) for j in range(4) if i < j]  # 6 pairs

# Schraudolph: int32(d*C1 + C2) bit-viewed as fp32 ~= exp(-d/2).
# C2 tuned for zero mean relative error under uniform exponent fraction.
C1 = float(np.float32(-0.5 * np.log2(np.e) * (1 << 23)))
C2 = float(np.float32((127.0 - 0.05752) * (1 << 23)))

# Four PSUM window tiles, one per consumer instruction (concurrent readers
# of one PSUM tile serialize in the tile framework, so every concurrently-
# running exp instruction gets its own tile):
#   T1 "A" [128,1024] xy[0:1024)    -> ScalarE exact exp
#   T2 "V" [128,1024] xy[1024:2048) -> DVE Schraudolph
#   T4 "A" [128, 640] xx: 16 d32 diag blocks (x w, diag exact on ACT),
#                         4 Q64 quarters, uppers 0-2        (x 2w)
#   T3 "V" [128, 448] xx: 8 q32 quarters, uppers 3-5        (x 2w)
_CACHE = {}


def _build_bass():
    import concourse.bass as bass
    import concourse.bacc as bacc
    import concourse.tile as tile
    import concourse.mybir as mybir

    f32 = mybir.dt.float32
    i32 = mybir.dt.int32
    bf16 = mybir.dt.bfloat16
    Exp = mybir.ActivationFunctionType.Exp
    Mult = mybir.AluOpType.mult
    Add = mybir.AluOpType.add
    nc = bacc.Bacc(
        "TRN2", target_bir_lowering=False, debug=False, num_devices=NCORES
    )

    XIN_d = nc.dram_tensor("XIN", (SPT, K, 4 * B), bf16, kind="ExternalInput").ap()
    ACC_d = nc.dram_tensor("ACC", (128, 2 * SPT), f32, kind="ExternalOutput").ap()

    with tile.TileContext(nc) as tc:
        with (
            tc.tile_pool(name="ins", bufs=4) as inpool,
            tc.tile_pool(name="ps", bufs=1, space="PSUM") as pspool,
            tc.tile_pool(name="es", bufs=3) as epool,
            tc.tile_pool(name="acc", bufs=1) as accpool,
        ):
            ones_t = accpool.tile([128, 1], f32)
            nc.gpsimd.memset(ones_t[:], 1.0)
            # window tiles (one consumer instruction each; concurrent readers
            # of one PSUM tile serialize, so windows are per-instruction):
            #  pa1 A [128,1024] xy mms 0-7
            #  pv1 V [128, 768] xy mms 8-13
            #  pa2 A [128, 704] xy mms 14-15 + d32(128,x w) + q32(64) +
            #                   Q64(128) + upper0(128)   (diag exact on ACT)
            #  pv2 V [128, 672] uppers 1-5 (640) + red cols [640:672)
            pa1 = pspool.tile([128, 896], f32, tag="pa1")
            pv1 = pspool.tile([128, 768], f32, tag="pv1")
            pa2 = pspool.tile([128, 832], f32, tag="pa2")
            pv2 = pspool.tile([128, 672], f32, tag="pv2")
            red_ps = pv2[:, 640 : 640 + 2 * SPT]
            # (tile, engine, exp cols, xy cols within those)
            segs = [(pa1, "A", 896, 896), (pv1, "V", 768, 768),
                    (pa2, "A", 832, 384), (pv2, "V", 640, 0)]
            etiles = []
            for s in range(SPT):
                xin_t = inpool.tile([K, 4 * B], bf16, tag="xin")
                if s == 0:
                    nc.sync.dma_start(xin_t[:, 0 : 2 * B], XIN_d[0][:, 0 : 2 * B])
                    nc.sync.dma_start(xin_t[:, 2 * B :], XIN_d[0][:, 2 * B :])
                else:
                    nc.sync.dma_start(xin_t[:], XIN_d[s])
                l_t = xin_t[:, 0 * B : 1 * B]
                ry_t = xin_t[:, 1 * B : 2 * B]
                ru_t = xin_t[:, 2 * B : 3 * B]
                rd_t = xin_t[:, 3 * B : 4 * B]

                # xy gram: 16 x 128-wide matmuls split across pa1/pv1/pa2
                for j in range(16):
                    pt, col = ((pa1, 128 * j) if j < 7 else
                               (pa2, 128 * (j - 7)) if j < 8 else
                               (pv1, 128 * (j - 8)) if j < 14 else
                               (pa2, 128 + 128 * (j - 13)))
                    nc.tensor.matmul(
                        pt[:, col : col + 128],
                        lhsT=l_t[:, 128 * (j // 4) : 128 * (j // 4 + 1)],
                        rhs=ry_t[:, 128 * (j % 4) : 128 * (j % 4 + 1)],
                        start=True,
                        stop=True,
                    )
                # pa2[256:384): d32 diag sub-blocks, 4-high (x w)
                for i in range(4):
                    for h in range(4):
                        d = 4 * i + h
                        base = 128 * i + 32 * h
                        nc.tensor.matmul(
                            pa2[32 * (d % 4) : 32 * (d % 4) + 32,
                                384 + 32 * (d // 4) : 384 + 32 * (d // 4) + 32],
                            lhsT=l_t[:, base : base + 32],
                            rhs=rd_t[:, base : base + 32],
                            start=True,
                            stop=True,
                            tile_position=(0, 32 * (d % 4)),
                        )
                # pa2[384:448): q32 quarters, 4-high (x 2w)
                for i in range(4):
                    for h in range(2):
                        q = 2 * i + h
                        base = 128 * i + 64 * h
                        nc.tensor.matmul(
                            pa2[32 * (q % 4) : 32 * (q % 4) + 32,
                                512 + 32 * (q // 4) : 544 + 32 * (q // 4)],
                            lhsT=l_t[:, base : base + 32],
                            rhs=ru_t[:, base + 32 : base + 64],
                            start=True,
                            stop=True,
                            tile_position=(0, 32 * (q % 4)),
                        )
                # pa2[448:576): Q64 quarters, 2-high (x 2w)
                for i in range(4):
                    half = 64 * (i % 2)
                    col = 576 + 64 * (i // 2)
                    nc.tensor.matmul(
                        pa2[half : half + 64, col : col + 64],
                        lhsT=l_t[:, 128 * i : 128 * i + 64],
                        rhs=ru_t[:, 128 * i + 64 : 128 * (i + 1)],
                        start=True,
                        stop=True,
                    )
                # upper-triangle blocks: k=0 -> pa2[576:704), k=1..5 -> pv2
                for k, (i, j) in enumerate(UPPER):
                    pt, col = (pa2, 704) if k == 0 else (pv2, 128 * (k - 1))
                    nc.tensor.matmul(
                        pt[:, col : col + 128],
                        lhsT=l_t[:, 128 * i : 128 * (i + 1)],
                        rhs=ru_t[:, 128 * j : 128 * (j + 1)],
                        start=True,
                        stop=True,
                    )

                # consumers: one instruction per window tile
                etl = []
                for idx, (pt, eng, w, _) in enumerate(segs):
                    e_t = epool.tile([128, w], f32, tag=f"e{idx}")
                    if eng == "A":
                        nc.scalar.activation(e_t[:], pt[:, 0:w], Exp, scale=-0.5)
                    else:
                        nc.vector.tensor_scalar(
                            e_t[:].bitcast(i32), pt[:, 0:w], C1, C2, Mult, Add)
                    etl.append(e_t)
                etiles.append(etl)

                if s > 0:
                    _reduce(nc, segs, etiles[s - 1], red_ps, ones_t, s - 1)
            _reduce(nc, segs, etiles[SPT - 1], red_ps, ones_t, SPT - 1)

            acc_sb = accpool.tile([128, 2 * SPT], f32)
            nc.vector.tensor_copy(acc_sb[:], red_ps[:])
            nc.sync.dma_start(ACC_d, acc_sb[:])

    nc.compile()
    return nc


def _reduce(nc, segs, etl, red_ps, ones_t, s):
    """Sum each exp tile into per-slice red columns (xy vs xx) via
    stationary-weights fp32 matmuls: ~4 PE cycles per 128-col chunk."""
    for want_xx in (False, True):  # xy first, then xx
        sel = []
        for (pt, eng, w, xyw), e_t in zip(segs, etl):
            lo = 0
            while lo < w:
                hi = min(lo + 128, w, xyw if lo < xyw else w)
                if (lo >= xyw) == want_xx:
                    sel.append((e_t, lo, hi))
                lo = hi
        for n, (e_t, lo, hi) in enumerate(sel):
            nc.tensor.matmul(
                red_ps[0 : hi - lo, 2 * s + int(want_xx) : 2 * s + int(want_xx) + 1],
                lhsT=e_t[:, lo:hi],
                rhs=ones_t[:],
                start=(n == 0),
                stop=(n == len(sel) - 1),
            )


def _split_hi_lo(v):
    hi = v.astype(BF16)
    lo = (v - hi.astype(np.float32)).astype(BF16)
    return hi, lo


def _rhs(neg2T, sq_shift):
    """neg2T: (SPT, C, B) bf16; sq_shift: (SPT, B) f32 -> (SPT, K, B) bf16."""
    R = np.empty((SPT, K, B), BF16)
    R[:, :C] = neg2T
    R[:, C], R[:, C + 1] = _split_hi_lo(sq_shift)
    R[:, C + 2] = np.asarray(1.0, BF16)
    R[:, C + 3] = np.asarray(1.0, BF16)
    return R


def _prep_core(xs, ys, w):
    """xs, ys: (B, SPT, C) f32; w: (SPT,) weights -> packed operands."""
    xb = xs.astype(BF16)
    yb = ys.astype(BF16)
    xT = np.ascontiguousarray(xb.transpose(1, 2, 0))  # (SPT, C, B)
    yT = np.ascontiguousarray(yb.transpose(1, 2, 0))
    nxT = (-2.0 * xT.astype(np.float32)).astype(BF16)  # exact 2x scale
    nyT = (-2.0 * yT.astype(np.float32)).astype(BF16)
    sqx = (xb.astype(np.float32) ** 2).sum(axis=2).T  # (SPT, B) f32
    sqy = (yb.astype(np.float32) ** 2).sum(axis=2).T

    L = np.empty((SPT, K, B), BF16)
    L[:, :C] = xT
    L[:, C] = np.asarray(1.0, BF16)
    L[:, C + 1] = np.asarray(1.0, BF16)
    L[:, C + 2], L[:, C + 3] = _split_hi_lo(sqx)

    c_u = 2.0 * w  # off-diag-pair blocks counted twice
    c_d = w
    c_y = w * (2.0 * (B - 1) / B)
    shift = lambda cs: (2.0 * np.log(cs))[:, None].astype(np.float32)
    RU = _rhs(nxT, sqx - shift(c_u))
    RD = _rhs(nxT, sqx - shift(c_d))
    RY = _rhs(nyT, sqy - shift(c_y))
    # packed to match the device layout: one DMA per slice
    return np.ascontiguousarray(np.concatenate([L, RY, RU, RD], axis=2))


def _run(x, y, trace=False, **kw):
    from concourse.bass_utils import run_bass_kernel_spmd

    if "nc" not in _CACHE:
        _CACHE["nc"] = _build_bass()
    nc = _CACHE["nc"]

    w = np.full(T, 2.0)
    w[0] = w[T - 1] = 1.0
    in_maps = []
    for c in range(NCORES):
        sl = slice(c * SPT, (c + 1) * SPT)
        in_maps.append({"XIN": _prep_core(x[:, sl, :], y[:, sl, :], w[sl])})

    return run_bass_kernel_spmd(
        nc, in_maps, list(range(NCORES)), trace=trace, **kw
    )


def _run_with_retries(x, y, trace=False, _trace_kw=None):
    """First execution of a freshly-loaded NEFF occasionally dies with
    NRT_EXEC_UNIT_UNRECOVERABLE; retry, resetting the jax backend in
    between, then fall back to a fresh subprocess."""
    import time as _time

    last = None
    for attempt in range(3):
        try:
            return _run(x, y, trace=trace, **(_trace_kw or {}))
        except Exception as e:  # noqa: BLE001
            last = e
            try:
                import jax

                jax.clear_caches()
                jax.clear_backends()
            except Exception:
                pass
            _time.sleep(2.0)
    # subprocess fallback: fresh process, fresh device session
    import os
    import pickle
    import subprocess
    import sys
    import tempfile

    kdir = os.path.dirname(os.path.abspath(__file__))
    with tempfile.TemporaryDirectory() as td:
        inp = os.path.join(td, "io.pkl")
        with open(inp, "wb") as f:
            pickle.dump({"x": x, "y": y}, f)
        code = (
            "import pickle, sys; sys.path.insert(0, %r); import kernel as km; "
            "d = pickle.load(open(%r, 'rb')); "
            "r = km.kernel(d['x'], d['y']); "
            "pickle.dump(r, open(%r, 'wb'))"
            % (kdir, inp, inp + ".out")
        )
        for attempt in range(2):
            p = subprocess.run(
                [sys.executable, "-c", code], capture_output=True, timeout=1800
            )
            if p.returncode == 0 and os.path.exists(inp + ".out"):
                with open(inp + ".out", "rb") as f:
                    return pickle.load(f)
    raise last


def kernel(x, y, _trace=False, _trace_kw=None):
    x = np.asarray(x, np.float32)
    y = np.asarray(y, np.float32)
    res = _run_with_retries(x, y, trace=_trace, _trace_kw=_trace_kw)
    if isinstance(res, np.floating | np.ndarray):
        return res  # came from the subprocess fallback, already reduced

    c_xx = 0.0
    c_xy = 0.0
    for c in range(NCORES):
        acc = np.asarray(res.results[c]["ACC"], np.float64)  # (128, 2*SPT)
        c_xy += acc[:, 0::2].sum()
        c_xx += acc[:, 1::2].sum()
    out = (c_xx - 512.0 * 254.0 - c_xy) / (B * (B - 1)) / 254.0
    if _trace:
        kernel.last_results = res
    return np.float32(out)
